# revision 11
# baseline (speedup 1.0000x reference)
"""Trainium2 Bass kernel: causal GQA attention.

Problem: B=2, Sq=Sk=2048, H=32, Hkv=8, D=128, fp32, causal + key-padding mask.

Sharding (8 cores): head-parallel. Core c takes q-heads [4c, 4c+4) for both
batches; those 4 heads share exactly one kv head (c) per batch. So each core
gets 8 (batch, head) pairs, with K/V loaded once per batch.

Matmuls run as float32r (fp32 rounded to e8m11, 1 PE cycle/row at free>=256 —
4x faster than plain fp32). Inputs are pre-rounded host-side (round-to-nearest
-even on the 2^-12 boundary) so DMA-loaded tiles are valid fp32r.

Device algorithm per (batch, head) pair (no-max-subtraction softmax — scaled
scores are ~N(0,1), so exp never overflows; masked entries get -1e4 bias
pre-exp and underflow to exactly 0, matching the reference numerics):

  for each q-group g of 512 queries (4 groups):
    for each pair of 128-wide key chunks (PSUM block [128, 2, 512]):
      S^T[j]  = K_j @ Q_g^T            (PE, [k=128, q=512] slices of block)
      diag chunks: add upper-triangle -1e4 bias (DVE); columns where the
        whole chunk is masked are never computed nor read downstream
      P^T     = exp(scale * S^T)       (ACT, one inst per full block,
                                        per-chunk sliced insts on diag)
      O^T    += V_j^T @ P^T[j]         (PE, accumulate [d=128, q=512])
      sums   += ones^T @ P^T[j]        (PE, accumulate [2, 512], dup rows)
    rsum = 1/sums                      (DVE)
    bcast = ones_col @ rsum            (PE outer product -> [128, 512])
    O^T_norm = O^T * bcast             (DVE, PSUM->SBUF)
    DMA O^T_norm out; host transposes [d, q] -> [q, d] while unsharding.

The key-padding mask is folded into the exp bias per key chunk (bias indexes
the partition axis = keys). The all-ones-mask fast path (the spec's fill)
uses a single zero bias, enabling block-batched exp; a non-trivial mask falls
back to per-chunk exp with the proper per-chunk bias.
"""

import math
import sys

import numpy as np

for _p in ("/opt/trn_rl_repo",):
    if _p not in sys.path:
        sys.path.append(_p)

import concourse.bass as bass
import concourse.tile as tile
from concourse import bacc, mybir
from concourse.bass import ts
from concourse.bass_utils import run_bass_kernel_spmd

B = 2
S = 2048
H = 32
HKV = 8
D = 128
N_CORES = 8
HPC = H // N_CORES  # q heads per core = 4
PAIRS = B * HPC  # 8 (batch, head) pairs per core
NG = S // 512  # 4 q-groups of 512 per pair
NCHUNK = S // 128  # 16 key chunks of 128
SCALE = 1.0 / math.sqrt(D)
NEG = -10000.0

F32 = mybir.dt.float32
F32R = mybir.dt.float32r
EXP = mybir.ActivationFunctionType.Exp


def round_fp32r(a: np.ndarray) -> np.ndarray:
    """Round fp32 to fp32r (e8m11): round-to-nearest-even at the 2^-12
    mantissa boundary, low 12 bits zeroed. Output is ordinary fp32 bits."""
    u = np.ascontiguousarray(a, dtype=np.float32).view(np.uint32)
    hi = u >> np.uint32(12)
    low = u & np.uint32(0xFFF)
    half = np.uint32(0x800)
    round_up = (low > half) | ((low == half) & ((hi & np.uint32(1)) == np.uint32(1)))
    out = ((hi + round_up.astype(np.uint32)) << np.uint32(12)).view(np.float32)
    return out


def build_module(uniform_mask: bool = True):
    nc = bacc.Bacc("TRN2", target_bir_lowering=False, debug=False, num_devices=1)

    qt = nc.dram_tensor("qt", [PAIRS, D, S], F32R, kind="ExternalInput").ap()
    kt = nc.dram_tensor("kt", [B, D, S], F32R, kind="ExternalInput").ap()
    v = nc.dram_tensor("v", [B, S, D], F32R, kind="ExternalInput").ap()
    tri = nc.dram_tensor("tri", [D, 128], F32, kind="ExternalInput").ap()
    pb = nc.dram_tensor("pb", [B, S], F32, kind="ExternalInput").ap()
    ot = nc.dram_tensor("ot", [PAIRS, NG, D, 512], F32, kind="ExternalOutput").ap()

    with tile.TileContext(nc) as tc:
        with (
            tc.tile_pool(name="consts", bufs=1) as consts,
            tc.tile_pool(name="kv", bufs=2) as kv_pool,
            tc.tile_pool(name="q", bufs=2) as q_pool,
            tc.tile_pool(name="pt", bufs=4) as pt_pool,
            tc.tile_pool(name="osb", bufs=3) as osb_pool,
            tc.tile_pool(name="small", bufs=4) as small_pool,
            tc.tile_pool(name="st_ps", bufs=3, space="PSUM") as st_pool,
            tc.tile_pool(name="ot_ps", bufs=1, space="PSUM") as ot_pool,
            tc.tile_pool(name="aux_ps", bufs=1, space="PSUM") as aux_pool,
        ):
            tri_sb = consts.tile([D, 128], F32)
            nc.sync.dma_start(tri_sb[:], tri[:])
            ones_f32 = consts.tile([D, 2], F32)
            nc.vector.memset(ones_f32[:], 1.0)
            ones_col = consts.tile([D, 2], F32R)  # [128,2] of 1.0
            nc.vector.tensor_copy(ones_col[:], ones_f32[:])


            for b in range(B):
                kt_sb = kv_pool.tile([D, S], F32R, tag="kt")
                nc.sync.dma_start(kt_sb[:], kt[b])
                v_sb = kv_pool.tile([D, NCHUNK, D], F32R, tag="v")
                nc.sync.dma_start(v_sb[:], v[b].rearrange("(j k) d -> k j d", k=128))
                pb_sb = kv_pool.tile([D, NCHUNK], F32, tag="pb")
                nc.sync.dma_start(pb_sb[:], pb[b].rearrange("(j k) -> k j", k=128))

                for h in range(HPC):
                    pair = b * HPC + h
                    qt_sb = q_pool.tile([D, S], F32R, tag="qt")
                    nc.sync.dma_start(qt_sb[:], qt[pair])

                    for g in range(NG):
                        nblk = 2 * (g + 1)  # 2-chunk blocks; last 2 are diag
                        nj = 4 * (g + 1)
                        ot_ps = ot_pool.tile([D, 512], F32)
                        sums_ps = aux_pool.tile([2, 512], F32, tag="aux")
                        for blk in range(nblk):
                            st = st_pool.tile([D, 2, 512], F32)
                            pt = pt_pool.tile([D, 2, 512], F32R)
                            qlos = []
                            for jj in range(2):
                                j = 2 * blk + jj
                                u = j - 4 * g  # >= 0 on diagonal chunks
                                qlo = max(0, 128 * u)
                                qlos.append(qlo)
                                nc.tensor.matmul(
                                    st[:, jj, qlo:],
                                    lhsT=kt_sb[:, ts(j, 128)],
                                    rhs=qt_sb[:, g * 512 + qlo : (g + 1) * 512],
                                    start=True,
                                    stop=True,
                                )
                                if u >= 0:
                                    # causal triangle within the diag block
                                    nc.vector.tensor_tensor(
                                        st[:, jj, qlo : qlo + 128],
                                        st[:, jj, qlo : qlo + 128],
                                        tri_sb[:],
                                        mybir.AluOpType.add,
                                    )
                            if uniform_mask and qlos == [0, 0]:
                                # one exp covering both chunks of the block
                                nc.scalar.activation(
                                    pt[:], st[:], EXP, scale=SCALE
                                )
                            else:
                                for jj in range(2):
                                    j = 2 * blk + jj
                                    qlo = qlos[jj]
                                    bias = (
                                        0.0
                                        if uniform_mask
                                        else pb_sb[:, j : j + 1]
                                    )
                                    nc.scalar.activation(
                                        pt[:, jj, qlo:],
                                        st[:, jj, qlo:],
                                        EXP,
                                        bias=bias,
                                        scale=SCALE,
                                    )
                            for jj in range(2):
                                j = 2 * blk + jj
                                qlo = qlos[jj]
                                nc.tensor.matmul(
                                    ot_ps[:, qlo:],
                                    lhsT=v_sb[:, j, :],
                                    rhs=pt[:, jj, qlo:],
                                    start=(j == 0),
                                    stop=(j == nj - 1),
                                )
                                nc.tensor.matmul(
                                    sums_ps[:, qlo:],
                                    lhsT=ones_col[:],
                                    rhs=pt[:, jj, qlo:],
                                    start=(j == 0),
                                    stop=(j == nj - 1),
                                )
                        rsum = small_pool.tile([1, 512], F32)
                        nc.vector.reciprocal(rsum[:], sums_ps[0:1, :])
                        # broadcast rsum across partitions via SWDGE DMA
                        # (replicating read, partition step 0) — keeps the
                        # normalizer off the PE/DVE critical path
                        rbc_sb = osb_pool.tile([D, 512], F32, tag="rbc")
                        nc.gpsimd.dma_start(
                            rbc_sb[:], rsum[:].to_broadcast([D, 512])
                        )
                        ot_sb = osb_pool.tile([D, 512], F32)
                        nc.vector.tensor_mul(ot_sb[:], ot_ps[:], rbc_sb[:])
                        nc.sync.dma_start(ot[pair, g], ot_sb[:])

    nc.compile()
    return nc


_NC = {}


def _get_nc(uniform_mask: bool = True):
    if uniform_mask not in _NC:
        _NC[uniform_mask] = build_module(uniform_mask)
    return _NC[uniform_mask]


def shard_inputs(q, kv, key_padding_mask):
    """Full inputs -> list of 8 per-core input maps (all contiguous fp32)."""
    q = np.asarray(q, dtype=np.float32)
    kv = np.asarray(kv, dtype=np.float32)
    mask = np.asarray(key_padding_mask)

    pbias = np.where(mask, np.float32(0.0), np.float32(NEG)).astype(np.float32)

    # in-tile causal triangle bias [k, q]: 0 if k <= q else -1e4
    kk = np.arange(128)[:, None]
    qq = np.arange(128)[None, :]
    tri = np.where(kk <= qq, np.float32(0.0), np.float32(NEG)).astype(np.float32)

    in_maps = []
    for c in range(N_CORES):
        qc = q[:, :, HPC * c : HPC * (c + 1), :]  # [B, S, 4, D]
        qt = round_fp32r(
            np.ascontiguousarray(np.transpose(qc, (0, 2, 3, 1))).reshape(PAIRS, D, S)
        )  # pair-major [b*4+h, D, S]
        kc = kv[:, :, 0, c, :]  # [B, S, D]
        vc = kv[:, :, 1, c, :]  # [B, S, D]
        ktc = round_fp32r(np.ascontiguousarray(np.transpose(kc, (0, 2, 1))))
        in_maps.append(
            {
                "qt": qt,
                "kt": ktc,
                "v": round_fp32r(vc),
                "tri": tri,
                "pb": pbias,
            }
        )
    return in_maps


def unshard_output(results):
    """Per-core 'ot' [PAIRS, NG, D, 512] -> full [B, S, H, D]."""
    out = np.empty((B, S, H, D), dtype=np.float32)
    for c in range(N_CORES):
        otc = results[c]["ot"]  # [8, 4, 128, 512]
        for pair in range(PAIRS):
            b, h = pair // HPC, HPC * c + pair % HPC
            # [NG, D, 512] -> [NG, 512, D] -> [S, D]
            out[b, :, h, :] = np.transpose(otc[pair], (0, 2, 1)).reshape(S, D)
    return out


def kernel(q, kv, key_padding_mask):
    uniform = bool(np.asarray(key_padding_mask).all())
    nc = _get_nc(uniform)
    in_maps = shard_inputs(q, kv, key_padding_mask)
    res = run_bass_kernel_spmd(nc, in_maps, core_ids=list(range(N_CORES)))
    return unshard_output(res.results)


# revision 13
# speedup vs baseline: 1.1124x; 1.1124x over previous
"""Trainium2 Bass kernel: causal GQA attention.

Problem: B=2, Sq=Sk=2048, H=32, Hkv=8, D=128, fp32, causal + key-padding mask.

Sharding (8 cores): head-parallel. Core c takes q-heads [4c, 4c+4) for both
batches; those 4 heads share exactly one kv head (c) per batch. So each core
gets 8 (batch, head) pairs, with K/V loaded once per batch.

Matmuls run as float32r (fp32 rounded to e8m11, 1 PE cycle/row at free>=256 —
4x faster than plain fp32). Inputs are pre-rounded host-side (round-to-nearest
-even on the 2^-12 boundary) so DMA-loaded tiles are valid fp32r.

Device algorithm per (batch, head) pair (no-max-subtraction softmax — scaled
scores are ~N(0,1), so exp never overflows; masked entries get -1e4 bias
pre-exp and underflow to exactly 0, matching the reference numerics):

  for each q-group g of 512 queries (4 groups):
    for each pair of 128-wide key chunks (PSUM block [128, 2, 512]):
      S^T[j]  = K_j @ Q_g^T            (PE, [k=128, q=512] slices of block)
      diag chunks: add upper-triangle -1e4 bias (DVE); columns where the
        whole chunk is masked are never computed nor read downstream
      P^T     = exp(scale * S^T)       (ACT, one inst per full block,
                                        per-chunk sliced insts on diag)
      O^T    += V_j^T @ P^T[j]         (PE, accumulate [d=128, q=512])
      sums   += ones^T @ P^T[j]        (PE, accumulate [2, 512], dup rows)
    rsum = 1/sums                      (DVE)
    bcast = ones_col @ rsum            (PE outer product -> [128, 512])
    O^T_norm = O^T * bcast             (DVE, PSUM->SBUF)
    DMA O^T_norm out; host transposes [d, q] -> [q, d] while unsharding.

The key-padding mask is folded into the exp bias per key chunk (bias indexes
the partition axis = keys). The all-ones-mask fast path (the spec's fill)
uses a single zero bias, enabling block-batched exp; a non-trivial mask falls
back to per-chunk exp with the proper per-chunk bias.
"""

import math
import sys

import numpy as np

for _p in ("/opt/trn_rl_repo",):
    if _p not in sys.path:
        sys.path.append(_p)

import concourse.bass as bass
import concourse.tile as tile
from concourse import bacc, mybir
from concourse.bass import ts
from concourse.bass_utils import run_bass_kernel_spmd

B = 2
S = 2048
H = 32
HKV = 8
D = 128
N_CORES = 8
HPC = H // N_CORES  # q heads per core = 4
PAIRS = B * HPC  # 8 (batch, head) pairs per core
NG = S // 512  # 4 q-groups of 512 per pair
NCHUNK = S // 128  # 16 key chunks of 128
SCALE = 1.0 / math.sqrt(D)
NEG = -10000.0

F32 = mybir.dt.float32
F32R = mybir.dt.float32r
EXP = mybir.ActivationFunctionType.Exp


def round_fp32r(a: np.ndarray) -> np.ndarray:
    """Round fp32 to fp32r (e8m11): round-to-nearest-even at the 2^-12
    mantissa boundary, low 12 bits zeroed. Output is ordinary fp32 bits."""
    u = np.ascontiguousarray(a, dtype=np.float32).view(np.uint32)
    hi = u >> np.uint32(12)
    low = u & np.uint32(0xFFF)
    half = np.uint32(0x800)
    round_up = (low > half) | ((low == half) & ((hi & np.uint32(1)) == np.uint32(1)))
    out = ((hi + round_up.astype(np.uint32)) << np.uint32(12)).view(np.float32)
    return out


def build_module(uniform_mask: bool = True):
    nc = bacc.Bacc("TRN2", target_bir_lowering=False, debug=False, num_devices=1)

    qt = nc.dram_tensor("qt", [PAIRS, D, S], F32R, kind="ExternalInput").ap()
    kt = nc.dram_tensor("kt", [B, D, S], F32R, kind="ExternalInput").ap()
    v = nc.dram_tensor("v", [B, S, D], F32R, kind="ExternalInput").ap()
    tri = nc.dram_tensor("tri", [D, 128], F32, kind="ExternalInput").ap()
    pb = nc.dram_tensor("pb", [B, S], F32, kind="ExternalInput").ap()
    ot = nc.dram_tensor("ot", [PAIRS, NG, D, 512], F32, kind="ExternalOutput").ap()

    with tile.TileContext(nc) as tc:
        with (
            tc.tile_pool(name="consts", bufs=1) as consts,
            tc.tile_pool(name="kv", bufs=2) as kv_pool,
            tc.tile_pool(name="q", bufs=2) as q_pool,
            tc.tile_pool(name="pt", bufs=4) as pt_pool,
            tc.tile_pool(name="osb", bufs=3) as osb_pool,
            tc.tile_pool(name="small", bufs=4) as small_pool,
            tc.tile_pool(name="st_ps", bufs=3, space="PSUM") as st_pool,
            tc.tile_pool(name="ot_ps", bufs=1, space="PSUM") as ot_pool,
            tc.tile_pool(name="aux_ps", bufs=1, space="PSUM") as aux_pool,
        ):
            tri_sb = consts.tile([D, 128], F32)
            nc.sync.dma_start(tri_sb[:], tri[:])
            ones_f32 = consts.tile([D, 2], F32)
            nc.vector.memset(ones_f32[:], 1.0)
            ones_col = consts.tile([D, 2], F32R)  # [128,2] of 1.0
            nc.vector.tensor_copy(ones_col[:], ones_f32[:])
            ones_row_f32 = consts.tile([1, D], F32)
            nc.vector.memset(ones_row_f32[:], 1.0)
            ones_row = consts.tile([1, D], F32R)  # [1,128] of 1.0
            nc.vector.tensor_copy(ones_row[:], ones_row_f32[:])


            for b in range(B):
                kt_sb = kv_pool.tile([D, S], F32R, tag="kt")
                nc.sync.dma_start(kt_sb[:], kt[b])
                v_sb = kv_pool.tile([D, NCHUNK, D], F32R, tag="v")
                nc.sync.dma_start(v_sb[:], v[b].rearrange("(j k) d -> k j d", k=128))
                pb_sb = kv_pool.tile([D, NCHUNK], F32, tag="pb")
                nc.sync.dma_start(pb_sb[:], pb[b].rearrange("(j k) -> k j", k=128))

                for h in range(HPC):
                    pair = b * HPC + h
                    qt_sb = q_pool.tile([D, S], F32R, tag="qt")
                    nc.sync.dma_start(qt_sb[:], qt[pair])

                    for g in range(NG):
                        nblk = 2 * (g + 1)  # 2-chunk blocks; last 2 are diag
                        nj = 4 * (g + 1)
                        ot_ps = ot_pool.tile([D, 512], F32)
                        sums_ps = aux_pool.tile([2, 512], F32, tag="aux")
                        for blk in range(nblk):
                            st = st_pool.tile([D, 2, 512], F32)
                            pt = pt_pool.tile([D, 2, 512], F32R)
                            qlos = []
                            for jj in range(2):
                                j = 2 * blk + jj
                                u = j - 4 * g  # >= 0 on diagonal chunks
                                qlo = max(0, 128 * u)
                                qlos.append(qlo)
                                nc.tensor.matmul(
                                    st[:, jj, qlo:],
                                    lhsT=kt_sb[:, ts(j, 128)],
                                    rhs=qt_sb[:, g * 512 + qlo : (g + 1) * 512],
                                    start=True,
                                    stop=True,
                                )
                                if u >= 0:
                                    # causal triangle within the diag block
                                    nc.vector.tensor_tensor(
                                        st[:, jj, qlo : qlo + 128],
                                        st[:, jj, qlo : qlo + 128],
                                        tri_sb[:],
                                        mybir.AluOpType.add,
                                    )
                            if uniform_mask and qlos == [0, 0]:
                                # one exp covering both chunks of the block
                                nc.scalar.activation(
                                    pt[:], st[:], EXP, scale=SCALE
                                )
                            else:
                                for jj in range(2):
                                    j = 2 * blk + jj
                                    qlo = qlos[jj]
                                    bias = (
                                        0.0
                                        if uniform_mask
                                        else pb_sb[:, j : j + 1]
                                    )
                                    nc.scalar.activation(
                                        pt[:, jj, qlo:],
                                        st[:, jj, qlo:],
                                        EXP,
                                        bias=bias,
                                        scale=SCALE,
                                    )
                            for jj in range(2):
                                j = 2 * blk + jj
                                qlo = qlos[jj]
                                nc.tensor.matmul(
                                    ot_ps[:, qlo:],
                                    lhsT=v_sb[:, j, :],
                                    rhs=pt[:, jj, qlo:],
                                    start=(j == 0),
                                    stop=(j == nj - 1),
                                )
                                nc.tensor.matmul(
                                    sums_ps[:, qlo:],
                                    lhsT=ones_col[:],
                                    rhs=pt[:, jj, qlo:],
                                    start=(j == 0),
                                    stop=(j == nj - 1),
                                )
                        rsum = small_pool.tile([1, 512], F32R)
                        with nc.allow_low_precision(
                            reason="fp32r normalizer: 2^-12 rel rounding is fine"
                        ):
                            nc.vector.reciprocal(rsum[:], sums_ps[0:1, :])
                        rbc_ps = aux_pool.tile([D, 512], F32, tag="aux")
                        nc.tensor.matmul(
                            rbc_ps[:],
                            lhsT=ones_row[:],
                            rhs=rsum[:],
                            start=True,
                            stop=True,
                        )
                        rbc_sb = osb_pool.tile([D, 512], F32, tag="rbc")
                        nc.vector.tensor_copy(rbc_sb[:], rbc_ps[:])
                        ot_sb = osb_pool.tile([D, 512], F32)
                        nc.vector.tensor_mul(ot_sb[:], ot_ps[:], rbc_sb[:])
                        nc.sync.dma_start(ot[pair, g], ot_sb[:])

    nc.compile()
    return nc


_NC = {}


def _get_nc(uniform_mask: bool = True):
    if uniform_mask not in _NC:
        _NC[uniform_mask] = build_module(uniform_mask)
    return _NC[uniform_mask]


def shard_inputs(q, kv, key_padding_mask):
    """Full inputs -> list of 8 per-core input maps (all contiguous fp32)."""
    q = np.asarray(q, dtype=np.float32)
    kv = np.asarray(kv, dtype=np.float32)
    mask = np.asarray(key_padding_mask)

    pbias = np.where(mask, np.float32(0.0), np.float32(NEG)).astype(np.float32)

    # in-tile causal triangle bias [k, q]: 0 if k <= q else -1e4
    kk = np.arange(128)[:, None]
    qq = np.arange(128)[None, :]
    tri = np.where(kk <= qq, np.float32(0.0), np.float32(NEG)).astype(np.float32)

    in_maps = []
    for c in range(N_CORES):
        qc = q[:, :, HPC * c : HPC * (c + 1), :]  # [B, S, 4, D]
        qt = round_fp32r(
            np.ascontiguousarray(np.transpose(qc, (0, 2, 3, 1))).reshape(PAIRS, D, S)
        )  # pair-major [b*4+h, D, S]
        kc = kv[:, :, 0, c, :]  # [B, S, D]
        vc = kv[:, :, 1, c, :]  # [B, S, D]
        ktc = round_fp32r(np.ascontiguousarray(np.transpose(kc, (0, 2, 1))))
        in_maps.append(
            {
                "qt": qt,
                "kt": ktc,
                "v": round_fp32r(vc),
                "tri": tri,
                "pb": pbias,
            }
        )
    return in_maps


def unshard_output(results):
    """Per-core 'ot' [PAIRS, NG, D, 512] -> full [B, S, H, D]."""
    out = np.empty((B, S, H, D), dtype=np.float32)
    for c in range(N_CORES):
        otc = results[c]["ot"]  # [8, 4, 128, 512]
        for pair in range(PAIRS):
            b, h = pair // HPC, HPC * c + pair % HPC
            # [NG, D, 512] -> [NG, 512, D] -> [S, D]
            out[b, :, h, :] = np.transpose(otc[pair], (0, 2, 1)).reshape(S, D)
    return out


def kernel(q, kv, key_padding_mask):
    uniform = bool(np.asarray(key_padding_mask).all())
    nc = _get_nc(uniform)
    in_maps = shard_inputs(q, kv, key_padding_mask)
    res = run_bass_kernel_spmd(nc, in_maps, core_ids=list(range(N_CORES)))
    return unshard_output(res.results)
